# revision 1
# baseline (speedup 1.0000x reference)
"""Trainium2 Bass kernel for nn_Attention_86268713108190.

7 independent attention "bands" over batch 8, n=512, d=512, 8 heads,
shared Wqkv/Wout. Sharding: data-parallel over batch — core c handles
batch index c (7 band-samples of [512, 512] each).

Per-core dataflow (per sample; all matmuls in float32r: HW-measured
~1.5e-4 matmul rel err at ~387 ns per N=512 matmul vs 853 ns for fp32):
  1. qkvT = Wqkv @ x^T    (lhsT = WqkvT chunks, rhs = x^T)      [e, n]
  2. v    = x @ Wv^T      (lhsT = x^T chunks,   rhs = WvT)      [n, ev]
     v_aug: per head 64 v-cols + a ones column (65) -> softmax
     denominator falls out of the AV matmul for free
  3. per head pair: S^T = k_h q_h^T (K=64), expS^T = exp(SCALE*S^T) on
     ACT (PSUM->SBUF, rounds to f32r; no max-subtraction needed --
     |SCALE*S| <~ 1.1 for this distribution), then
     O_aug^T[65, n] = v_aug.T @ expS^T accumulated over j-tiles;
     row 64 = softmax denominator. Softmax reduction runs over the
     PSUM partition axis via the ones column, so no transposes at all.
  4. normalize tail (fully lagged one head pair, emitted after the
     next pair's S+exp so nothing blocks the in-order PE/ACT streams):
     1/d = exp(-ln d) on ACT (ln+exp share one table set; DVE's
     iterative reciprocal is slow, and reciprocal_approx_fast returns
     garbage on HW despite passing CoreSim), bounced through a DRAM
     scratch tile and broadcast to 64 partitions by a stride-0
     DRAM-source DMA (SBUF stride-0 APs are illegal but DRAM-source
     ones lower fine); two DVE multiplies -> OT [d, n].
  5. out = O @ Wout^T + bias  (lhsT = OT chunks, rhs = WoutT).

Whole-output HW accuracy vs fp32 reference: rel err ~2.9e-4.
Steady-state HW time per core (7 bands): ~500 us in the final A/B
session (measured by For_i repeat differencing; session-to-session
terminal variance is ~+-5%). no_tail ablation floor is ~364 us; the
residual gap is the normalize tail's DVE reciprocal + cross-engine
chain, which measured equal across GpSimd/PE-matmul/DMA broadcast
mechanisms, lagged or not. HW ablations: QKV+out-proj alone run at
~133 us, attention S/exp/AV adds ~170 us, and the softmax-normalize
tail adds the rest -- its cross-engine latency chain is the main
non-PE cost; PE-stream (mask-matmul) and lagged variants measured
slower than the off-stream GpSimd broadcast.
"""

import contextlib
import sys

if '/opt/trn_rl_repo' not in sys.path:
    sys.path.insert(0, '/opt/trn_rl_repo')

import numpy as np

P = 128
MM_DTYPE = "f32r"
NSEQ = 512
D = 512
H = 8
DH = 64
NBANDS = 7
NCORES = 8
SCALE = D ** -0.5

_cached = None


def _emit_band(ctx, s, xt):
    """Emit one band's compute. `xt` is the (already DMA'd) x^T tile."""
    nc, f32, f32r, Exp = ctx["nc"], ctx["f32"], ctx["f32r"], ctx["Exp"]
    wq_sb, wo_sb, bias_sb = ctx["wq_sb"], ctx["wo_sb"], ctx["bias_sb"]
    out = ctx["out"]
    pl = ctx["pools"]

    # --- QKV projections -> qkvT layout for q,k ---
    qk_sb = pl["qk"].tile([P, 8, NSEQ], f32r, tag="qk")
    for et in (0, 4, 1, 5, 2, 6, 3, 7):
        ps = pl["psproj"].tile([P, NSEQ], f32, tag="psproj")
        for kt in range(4):
            nc.tensor.matmul(
                ps[:], wq_sb[:, kt, et * P:(et + 1) * P], xt[:, kt, :],
                start=(kt == 0), stop=(kt == 3))
        nc.vector.tensor_copy(qk_sb[:, et, :], ps[:])

    # --- V projection -> row-major v_aug with ones column ---
    v_aug = pl["v"].tile([P, 4, H, DH + 1], f32r, tag="vaug")
    for nt in range(4):
        ps = pl["psproj"].tile([P, NSEQ], f32, tag="psproj")
        for kt in range(4):
            nc.tensor.matmul(
                ps[:], xt[:, kt, nt * P:(nt + 1) * P],
                wq_sb[:, kt, 2 * D:3 * D],
                start=(kt == 0), stop=(kt == 3))
        nc.vector.tensor_copy(
            v_aug[:, nt, :, 0:DH],
            ps[:].rearrange("p (h dh) -> p h dh", h=H))
        ones_slice = v_aug[:, nt, :, DH:DH + 1]
        if ctx["mm_dtype"] == "f32r":
            ones_slice = ones_slice.bitcast(f32)
        nc.vector.memset(ones_slice, 1.0)

    # --- attention per head pair (2g, 2g+1) ---
    # Three emission orders were measured on HW; "split" (all S+exp of a
    # pair, then its AVs) was fastest by a small margin.
    ot_sb = pl["ot"].tile([P, 4, NSEQ], f32r, tag="ot")
    es_store = {}

    def s_phase(g):
        es_list = []
        for jt in range(4):
            ps_s0 = pl["pss"].tile([P, NSEQ], f32, tag="pss")
            ps_s1 = pl["pss"].tile([P, NSEQ], f32, tag="pss")
            nc.tensor.matmul(
                ps_s0[:],
                qk_sb[0:DH, 4 + g, jt * P:(jt + 1) * P],
                qk_sb[0:DH, g, :], start=True, stop=True)
            nc.tensor.matmul(
                ps_s1[:],
                qk_sb[DH:P, 4 + g, jt * P:(jt + 1) * P],
                qk_sb[DH:P, g, :], start=True, stop=True,
                tile_position=(DH, 0))
            es = pl["es"].tile([P, 2, NSEQ], f32r, tag="es")
            nc.scalar.activation(es[:, 0, :], ps_s0[:], Exp, scale=SCALE)
            nc.scalar.activation(es[:, 1, :], ps_s1[:], Exp, scale=SCALE)
            es_list.append(es)
        es_store[g] = es_list

    tail_store = {}

    def av_phase(g):
        es_list = es_store.pop(g)
        if ctx["ablate"] == "no_av":
            nc.vector.tensor_copy(ot_sb[:, g, :], es_list[0][:, 0, :])
            return
        ps_o0 = pl["pso"].tile([DH + 1, NSEQ], f32, tag="pso")
        ps_o1 = pl["pso"].tile([DH + 1, NSEQ], f32, tag="pso")
        for jt in range(4):
            nc.tensor.matmul(
                ps_o0[:], v_aug[:, jt, 2 * g, :], es_list[jt][:, 0, :],
                start=(jt == 0), stop=(jt == 3))
            nc.tensor.matmul(
                ps_o1[:], v_aug[:, jt, 2 * g + 1, :], es_list[jt][:, 1, :],
                start=(jt == 0), stop=(jt == 3))
        if ctx["ablate"] == "no_tail":
            nc.vector.tensor_copy(ot_sb[0:DH, g, :], ps_o0[0:DH, :])
            nc.vector.tensor_copy(ot_sb[DH:P, g, :], ps_o1[0:DH, :])
            return
        if ctx["tail"] == "pbcast":
            rcc = pl["r"].tile([1, 2 * NSEQ], f32, tag="rcc")
            nc.vector.reciprocal(rcc[0:1, 0:NSEQ], ps_o0[DH:DH + 1, :])
            nc.vector.reciprocal(rcc[0:1, NSEQ:2 * NSEQ],
                                 ps_o1[DH:DH + 1, :])
            rb = pl["r"].tile([DH, 2 * NSEQ], f32, tag="rb")
            nc.gpsimd.partition_broadcast(rb[:], rcc[:])
            nc.vector.tensor_mul(ot_sb[0:DH, g, :], ps_o0[0:DH, :],
                                 rb[:, 0:NSEQ])
            nc.vector.tensor_mul(ot_sb[DH:P, g, :], ps_o1[0:DH, :],
                                 rb[:, NSEQ:2 * NSEQ])
            return
        if ctx["tail"] == "dma":
            # whole tail is computed lagged in tail_finish, after the
            # NEXT pair's S+exp, so the ACT Ln/Exp reciprocal never
            # blocks the softmax exps in ACT's in-order stream.
            tail_store[g] = (ps_o0, ps_o1, None, "lagall")
            return
        # lagged PE mask-matmul tail: recips now, broadcast+mults under
        # the next pair's S matmuls.
        rc0 = pl["r"].tile([1, NSEQ], f32, tag="rc0")
        rc1 = pl["r"].tile([1, NSEQ], f32, tag="rc1")
        nc.vector.reciprocal(rc0[:], ps_o0[DH:DH + 1, :])
        nc.vector.reciprocal(rc1[:], ps_o1[DH:DH + 1, :])
        tail_store[g] = (ps_o0, ps_o1, rc0, rc1)

    def tail_finish(g):
        if g not in tail_store:
            return
        ps_o0, ps_o1, rc0, rc1 = tail_store.pop(g)
        if rc1 == "lagall":
            # 1/d = exp(-ln d) on ACT (ln+exp share the
            # natural_log_exp_and_others table set); DRAM-bounce DMA
            # broadcast; DVE multiplies.
            lg = pl["r"].tile([1, 2 * NSEQ], f32, tag="lg")
            Ln = ctx["Ln"]
            nc.scalar.activation(lg[0:1, 0:NSEQ], ps_o0[DH:DH + 1, :], Ln)
            nc.scalar.activation(lg[0:1, NSEQ:2 * NSEQ],
                                 ps_o1[DH:DH + 1, :], Ln)
            rcc = pl["r"].tile([1, 2 * NSEQ], f32, tag="rcc")
            nc.scalar.activation(rcc[:], lg[:], Exp, scale=-1.0)
            dr = pl["dram"].tile([1, 2 * NSEQ], f32, tag="dr")
            nc.sync.dma_start(dr[:], rcc[:])
            rb = pl["r"].tile([DH, 2 * NSEQ], f32, tag="rb")
            nc.sync.dma_start(rb[:], dr[:].to_broadcast((DH, 2 * NSEQ)))
            nc.vector.tensor_mul(ot_sb[0:DH, g, :], ps_o0[0:DH, :],
                                 rb[:, 0:NSEQ])
            nc.vector.tensor_mul(ot_sb[DH:P, g, :], ps_o1[0:DH, :],
                                 rb[:, NSEQ:2 * NSEQ])
            return
        if rc1 == "dma":  # rc0 is a DRAM [1, 2*NSEQ] recip row
            rb = pl["r"].tile([DH, 2 * NSEQ], f32, tag="rb")
            nc.sync.dma_start(rb[:], rc0[:].to_broadcast((DH, 2 * NSEQ)))
            nc.vector.tensor_mul(ot_sb[0:DH, g, :], ps_o0[0:DH, :],
                                 rb[:, 0:NSEQ])
            nc.vector.tensor_mul(ot_sb[DH:P, g, :], ps_o1[0:DH, :],
                                 rb[:, NSEQ:2 * NSEQ])
            return
        if rc1 is None:  # pblag: rc0 is the merged [1, 1024] recip row
            rb = pl["r"].tile([DH, 2 * NSEQ], f32, tag="rb")
            nc.gpsimd.partition_broadcast(rb[:], rc0[:])
            nc.vector.tensor_mul(ot_sb[0:DH, g, :], ps_o0[0:DH, :],
                                 rb[:, 0:NSEQ])
            nc.vector.tensor_mul(ot_sb[DH:P, g, :], ps_o1[0:DH, :],
                                 rb[:, NSEQ:2 * NSEQ])
            return
        rb_ps = pl["psproj"].tile([P, NSEQ], f32, tag="psproj")
        nc.tensor.matmul(rb_ps[:], ctx["maskA"][:], rc0[:],
                         start=True, stop=False)
        nc.tensor.matmul(rb_ps[:], ctx["maskB"][:], rc1[:],
                         start=False, stop=True)
        rb_sb = pl["r"].tile([P, NSEQ], f32, tag="rbsb")
        nc.vector.tensor_copy(rb_sb[:], rb_ps[:])
        nc.vector.tensor_mul(ot_sb[0:DH, g, :], ps_o0[0:DH, :],
                             rb_sb[0:DH, :])
        nc.vector.tensor_mul(ot_sb[DH:P, g, :], ps_o1[0:DH, :],
                             rb_sb[DH:P, :])

    def interleaved_pair(g):
        ps_o0 = pl["pso"].tile([DH + 1, NSEQ], f32, tag="pso")
        ps_o1 = pl["pso"].tile([DH + 1, NSEQ], f32, tag="pso")
        for jt in range(4):
            ps_s0 = pl["pss"].tile([P, NSEQ], f32, tag="pss")
            ps_s1 = pl["pss"].tile([P, NSEQ], f32, tag="pss")
            nc.tensor.matmul(
                ps_s0[:],
                qk_sb[0:DH, 4 + g, jt * P:(jt + 1) * P],
                qk_sb[0:DH, g, :], start=True, stop=True)
            nc.tensor.matmul(
                ps_s1[:],
                qk_sb[DH:P, 4 + g, jt * P:(jt + 1) * P],
                qk_sb[DH:P, g, :], start=True, stop=True,
                tile_position=(DH, 0))
            es = pl["es"].tile([P, 2, NSEQ], f32r, tag="es")
            nc.scalar.activation(es[:, 0, :], ps_s0[:], Exp, scale=SCALE)
            nc.scalar.activation(es[:, 1, :], ps_s1[:], Exp, scale=SCALE)
            nc.tensor.matmul(
                ps_o0[:], v_aug[:, jt, 2 * g, :], es[:, 0, :],
                start=(jt == 0), stop=(jt == 3))
            nc.tensor.matmul(
                ps_o1[:], v_aug[:, jt, 2 * g + 1, :], es[:, 1, :],
                start=(jt == 0), stop=(jt == 3))
        rc0 = pl["r"].tile([1, NSEQ], f32, tag="rc0")
        rc1 = pl["r"].tile([1, NSEQ], f32, tag="rc1")
        nc.vector.reciprocal(rc0[:], ps_o0[DH:DH + 1, :])
        nc.vector.reciprocal(rc1[:], ps_o1[DH:DH + 1, :])
        rb0 = pl["r"].tile([DH, NSEQ], f32, tag="rb0")
        rb1 = pl["r"].tile([DH, NSEQ], f32, tag="rb1")
        nc.gpsimd.partition_broadcast(rb0[:], rc0[:])
        nc.gpsimd.partition_broadcast(rb1[:], rc1[:])
        nc.vector.tensor_mul(ot_sb[0:DH, g, :], ps_o0[0:DH, :], rb0[:])
        nc.vector.tensor_mul(ot_sb[DH:P, g, :], ps_o1[0:DH, :], rb1[:])

    if ctx["ablate"] == "no_attn":
        nc.vector.tensor_copy(ot_sb[:], qk_sb[:, 0:4, :])
    elif ctx["ablate"] == "no_exp":
        for g in range(4):
            for jt in range(4):
                ps_s0 = pl["pss"].tile([P, NSEQ], f32, tag="pss")
                ps_s1 = pl["pss"].tile([P, NSEQ], f32, tag="pss")
                nc.tensor.matmul(
                    ps_s0[:], qk_sb[0:DH, 4 + g, jt * P:(jt + 1) * P],
                    qk_sb[0:DH, g, :], start=True, stop=True)
                nc.tensor.matmul(
                    ps_s1[:], qk_sb[DH:P, 4 + g, jt * P:(jt + 1) * P],
                    qk_sb[DH:P, g, :], start=True, stop=True,
                    tile_position=(DH, 0))
                es = pl["es"].tile([P, 2, NSEQ], f32r, tag="es")
                nc.vector.tensor_copy(es[:, 0, :], ps_s0[:])
                nc.vector.tensor_copy(es[:, 1, :], ps_s1[:])
                es_store.setdefault(g, []).append(es)
            av_phase(g)
    elif ctx["pipe"] == "pipe":
        s_phase(0)
        for g in range(1, 4):
            s_phase(g)
            av_phase(g - 1)
        av_phase(3)
    elif ctx["pipe"] == "split":
        for g in range(4):
            s_phase(g)
            tail_finish(g - 1)
            av_phase(g)
        tail_finish(3)
    else:  # "v2": exp and AV interleaved per j-tile
        for g in range(4):
            interleaved_pair(g)

    # --- output projection + bias ---
    for nt in range(4):
        ps = pl["psproj"].tile([P, NSEQ], f32, tag="psproj")
        for kt in range(4):
            nc.tensor.matmul(
                ps[:], ot_sb[:, kt, nt * P:(nt + 1) * P], wo_sb[:, kt, :],
                start=(kt == 0), stop=(kt == 3))
        ob = pl["ob"].tile([P, D], f32, tag="ob")
        nc.vector.tensor_add(ob[:], ps[:], bias_sb[:])
        nc.sync.dma_start(
            out[s].rearrange("(no ni) e -> ni no e", ni=P)[:, nt, :], ob[:])


def build_kernel(nbands=NBANDS, repeat=1, mm_dtype=MM_DTYPE, pipe="split", ablate="", tail="dma"):
    import concourse.mybir as mybir
    import concourse.tile as tile
    from concourse import bacc
    from concourse import library_config

    f32 = mybir.dt.float32
    f32r = (mybir.dt.float32r if mm_dtype == "f32r" else mybir.dt.bfloat16)
    Exp = mybir.ActivationFunctionType.Exp
    Ln = mybir.ActivationFunctionType.Ln

    nc = bacc.Bacc("TRN2", target_bir_lowering=False, debug=False,
                   num_devices=NCORES)

    xT = nc.dram_tensor("xT", [nbands, D, NSEQ], f32r, kind="ExternalInput").ap()
    wqkvT = nc.dram_tensor("wqkvT", [D, 3 * D], f32r, kind="ExternalInput").ap()
    woutT = nc.dram_tensor("woutT", [D, D], f32r, kind="ExternalInput").ap()
    biasb = nc.dram_tensor("biasb", [P, D], f32, kind="ExternalInput").ap()
    out = nc.dram_tensor("out", [nbands, NSEQ, D], f32, kind="ExternalOutput").ap()

    nc.gpsimd.load_library(library_config.attn)

    with tile.TileContext(nc) as tc:
        with (
            tc.tile_pool(name="weights", bufs=1) as wpool,
            tc.tile_pool(name="x", bufs=3) as xpool,
            tc.tile_pool(name="qk", bufs=2) as qkpool,
            tc.tile_pool(name="v", bufs=2) as vpool,
            tc.tile_pool(name="ot", bufs=2) as otpool,
            tc.tile_pool(name="es", bufs=8) as spool,
            tc.tile_pool(name="r", bufs=3) as rpool,
            tc.tile_pool(name="ob", bufs=3) as outpool,
            tc.tile_pool(name="dram", bufs=3, space="DRAM") as drampool,
            tc.tile_pool(name="psproj", bufs=2, space="PSUM") as psproj,
            tc.tile_pool(name="pss", bufs=2, space="PSUM") as pss,
            tc.tile_pool(name="pso", bufs=4, space="PSUM") as pso,
        ):
            # weights: split wq by k-chunk so the first matmuls can start
            # as soon as their chunk lands
            wq_sb = wpool.tile([P, 4, 3 * D], f32r)
            wo_sb = wpool.tile([P, 4, D], f32r)
            bias_sb = wpool.tile([P, D], f32)
            maskA = wpool.tile([1, P], f32)
            maskB = wpool.tile([1, P], f32)
            nc.vector.memset(maskA[:], 0.0)
            nc.vector.memset(maskB[:], 0.0)
            nc.vector.memset(maskA[0:1, 0:DH], 1.0)
            nc.vector.memset(maskB[0:1, DH:P], 1.0)
            wq_r = wqkvT.rearrange("(ko ki) e -> ki ko e", ki=P)
            for kt in range(4):
                nc.sync.dma_start(wq_sb[:, kt, :], wq_r[:, kt, :])
            nc.sync.dma_start(wo_sb[:], woutT.rearrange("(ko ki) e -> ki ko e", ki=P))
            nc.sync.dma_start(bias_sb[:], biasb[:])

            ctx = {
                "nc": nc, "f32": f32, "f32r": f32r, "Exp": Exp, "Ln": Ln,
                "mm_dtype": mm_dtype, "pipe": pipe, "ablate": ablate, "tail": tail,
                "maskA": maskA, "maskB": maskB,
                "wq_sb": wq_sb, "wo_sb": wo_sb, "bias_sb": bias_sb,
                "out": out,
                "pools": {
                    "qk": qkpool, "v": vpool, "ot": otpool, "es": spool,
                    "r": rpool, "ob": outpool, "psproj": psproj,
                    "dram": drampool,
                    "pss": pss, "pso": pso,
                },
            }

            def load_x(s):
                xt = xpool.tile([P, 4, NSEQ], f32r, tag="xt")
                nc.sync.dma_start(
                    xt[:], xT[s].rearrange("(ko ki) n -> ki ko n", ki=P))
                return xt

            rep_ctx = (tc.For_i(0, repeat, 1,
                                hint_engines=(mybir.EngineType.PE,
                                              mybir.EngineType.Activation,
                                              mybir.EngineType.DVE))
                       if repeat > 1 else contextlib.nullcontext())
            with rep_ctx:
                # prefetch x one band ahead
                xt_next = load_x(0)
                for s in range(nbands):
                    xt = xt_next
                    if s + 1 < nbands:
                        xt_next = load_x(s + 1)
                    _emit_band(ctx, s, xt)

    nc.compile()
    return nc


def _get_nc():
    global _cached
    if _cached is None:
        _cached = build_kernel()
    return _cached


def make_in_maps(x, x_delta, x_theta, x_alpha, x_beta, x_gamma, x_upper,
                 Wqkv, Wout, bout, mm_dtype=MM_DTYPE):
    if mm_dtype == "f32r":
        cast_dt = np.float32
    else:
        import ml_dtypes
        cast_dt = ml_dtypes.bfloat16
    xs = np.stack([np.asarray(a, dtype=np.float32) for a in
                   (x, x_delta, x_theta, x_alpha, x_beta, x_gamma, x_upper)],
                  axis=0)  # [7, b, n, d]
    xsT = np.ascontiguousarray(xs.transpose(1, 0, 3, 2).astype(cast_dt))
    wqkvT = np.ascontiguousarray(np.asarray(Wqkv, np.float32).T.astype(cast_dt))
    woutT = np.ascontiguousarray(np.asarray(Wout, np.float32).T.astype(cast_dt))
    biasb = np.ascontiguousarray(
        np.broadcast_to(np.asarray(bout, np.float32)[None, :], (P, D)))
    return [
        {"xT": xsT[c], "wqkvT": wqkvT, "woutT": woutT, "biasb": biasb}
        for c in range(NCORES)
    ]


def kernel(x, x_delta, x_theta, x_alpha, x_beta, x_gamma, x_upper,
           Wqkv, Wout, bout):
    from concourse.bass_utils import run_bass_kernel_spmd

    nc = _get_nc()
    in_maps = make_in_maps(x, x_delta, x_theta, x_alpha, x_beta, x_gamma,
                           x_upper, Wqkv, Wout, bout)
    res = run_bass_kernel_spmd(nc, in_maps, core_ids=list(range(NCORES)))
    full = np.empty((NBANDS, NCORES, NSEQ, D), dtype=np.float32)
    for c in range(NCORES):
        full[:, c] = res.results[c]["out"]
    return tuple(full[i] for i in range(NBANDS))



# revision 3
# speedup vs baseline: 3.6884x; 3.6884x over previous
"""Trainium2 Bass kernel for nn_Attention_86268713108190.

7 independent attention "bands" over batch 8, n=512, d=512, 8 heads,
shared Wqkv/Wout. Sharding: data-parallel over batch — core c handles
batch index c (7 band-samples of [512, 512] each).

v2: software-pipelined band schedule built around keeping the PE
(tensor engine) continuously fed so the HAM clock gate stays at
2.4 GHz (it throttles to 1.2 GHz after ~3.4 us of idle):

  - Per band: QKV/V projections (12 groups x 4 matmuls, f32r), S^T
    matmuls per head pair as row-tiled concurrent pairs
    (tile_position (0,0)/(64,0), K=64), batched exp on ACT over a
    2-bank PSUM tile [128, 2x512], AV with a ones column appended to V
    so the softmax denominator falls out of the AV matmul (row 64).
  - The attention phase is ACT-paced (exp), so the NEXT band's
    projection matmuls and the PREVIOUS band's out-projection are
    interleaved as fillers between S units to keep PE busy.
  - Softmax normalize: AV PSUM is evacuated to SBUF ([65, 512] copies,
    split ACT/DVE), denominator rows DMA to a DRAM scratch, ONE
    [128, 8x4] gather + Ln + Exp(scale=-1) computes all 8 heads'
    reciprocals per band in lane-parallel row-major form (instead of
    lane-starved [1, N] ops), then a stride-0 DRAM broadcast DMA and
    2 DVE multiplies per pair normalize O^T. Out-projection of band s
    runs as filler inside band s+1, so nothing on the PE ever waits
    for the normalize chain.

Whole-output HW accuracy vs fp32 reference: rel err ~3e-4 (f32r
matmuls everywhere, same numerics as v1).
"""

import contextlib
import sys

if '/opt/trn_rl_repo' not in sys.path:
    sys.path.insert(0, '/opt/trn_rl_repo')

import numpy as np

P = 128
MM_DTYPE = "f32r"
NSEQ = 512
D = 512
H = 8
DH = 64
NBANDS = 7
NCORES = 8
SCALE = D ** -0.5

_cached = None


def build_kernel(nbands=NBANDS, repeat=1, mm_dtype=MM_DTYPE):
    import concourse.mybir as mybir
    import concourse.tile as tile
    from concourse import bacc
    from concourse import library_config

    f32 = mybir.dt.float32
    f32r = (mybir.dt.float32r if mm_dtype == "f32r" else mybir.dt.bfloat16)
    Exp = mybir.ActivationFunctionType.Exp
    Ln = mybir.ActivationFunctionType.Ln
    Copy = mybir.ActivationFunctionType.Copy

    nc = bacc.Bacc("TRN2", target_bir_lowering=False, debug=False,
                   num_devices=NCORES)

    xT = nc.dram_tensor("xT", [nbands, D, NSEQ], f32r, kind="ExternalInput").ap()
    wqkvT = nc.dram_tensor("wqkvT", [D, 3 * D], f32r, kind="ExternalInput").ap()
    woutT = nc.dram_tensor("woutT", [D, D], f32r, kind="ExternalInput").ap()
    biasb = nc.dram_tensor("biasb", [P, D], f32, kind="ExternalInput").ap()
    out = nc.dram_tensor("out", [nbands, NSEQ, D], f32, kind="ExternalOutput").ap()

    nc.gpsimd.load_library(library_config.attn)

    with tile.TileContext(nc) as tc:
        with (
            tc.tile_pool(name="weights", bufs=1) as wpool,
            tc.tile_pool(name="x", bufs=3) as xpool,
            tc.tile_pool(name="qk", bufs=2) as qkpool,
            tc.tile_pool(name="v", bufs=2) as vpool,
            tc.tile_pool(name="ot", bufs=2) as otpool,
            tc.tile_pool(name="es", bufs=8) as espool,
            tc.tile_pool(name="oraw", bufs=5) as opool,
            tc.tile_pool(name="r", bufs=2) as rpool,
            tc.tile_pool(name="rb", bufs=3) as rbpool,
            tc.tile_pool(name="ob", bufs=3) as obpool,
            tc.tile_pool(name="dram", bufs=2, space="DRAM") as drampool,
            tc.tile_pool(name="psproj", bufs=2, space="PSUM") as psproj,
            tc.tile_pool(name="pss", bufs=2, space="PSUM") as pssp,
            tc.tile_pool(name="pso", bufs=2, space="PSUM") as psop,
        ):
            wq_sb = wpool.tile([P, 4, 3 * D], f32r)
            wo_sb = wpool.tile([P, 4, D], f32r)
            bias_sb = wpool.tile([P, D], f32)
            wq_r = wqkvT.rearrange("(ko ki) e -> ki ko e", ki=P)
            for kt in range(4):
                nc.sync.dma_start(wq_sb[:, kt, :], wq_r[:, kt, :])
            nc.sync.dma_start(wo_sb[:], woutT.rearrange("(ko ki) e -> ki ko e", ki=P))
            nc.sync.dma_start(bias_sb[:], biasb[:])

            # per-band live tiles (keyed by band index)
            xt = {}
            qk = {}
            va = {}
            ot = {}
            oraw = {}
            es = {}
            dD = {}
            rD = {}

            ET_ORDER = (0, 4, 1, 5, 2, 6, 3, 7)

            def load_x(s):
                t = xpool.tile([P, 4, NSEQ], f32r, tag="xt", name="xt")
                nc.sync.dma_start(
                    t[:], xT[s].rearrange("(ko ki) n -> ki ko n", ki=P))
                xt[s] = t

            def proj_unit(s, k):
                """k in 0..7: q,k column groups; k in 8..11: v row groups."""
                if k == 0:
                    qk[s] = qkpool.tile([P, 8, NSEQ], f32r, tag="qk", name="qk")
                if k == 8:
                    va[s] = vpool.tile([P, 4, H, DH + 1], f32r, tag="va",
                                       name="va")
                if k < 8:
                    et = ET_ORDER[k]
                    ps = psproj.tile([P, NSEQ], f32, tag="psproj", name="psp")
                    for kt in range(4):
                        nc.tensor.matmul(
                            ps[:], wq_sb[:, kt, et * P:(et + 1) * P],
                            xt[s][:, kt, :], start=(kt == 0), stop=(kt == 3))
                    nc.vector.tensor_copy(qk[s][:, et, :], ps[:])
                else:
                    nt = k - 8
                    ps = psproj.tile([P, NSEQ], f32, tag="psproj", name="psp")
                    for kt in range(4):
                        nc.tensor.matmul(
                            ps[:], xt[s][:, kt, nt * P:(nt + 1) * P],
                            wq_sb[:, kt, 2 * D:3 * D],
                            start=(kt == 0), stop=(kt == 3))
                    nc.vector.tensor_copy(
                        va[s][:, nt, :, 0:DH],
                        ps[:].rearrange("p (h dh) -> p h dh", h=H))
                    ones_slice = va[s][:, nt, :, DH:DH + 1]
                    if mm_dtype == "f32r":
                        ones_slice = ones_slice.bitcast(f32)
                    nc.vector.memset(ones_slice, 1.0)

            def s_unit(s, g, jt):
                """S^T for head pair (2g, 2g+1), j-tile jt, + batched exp."""
                ps2 = pssp.tile([P, 2, NSEQ], f32, tag="pss", name="pss")
                nc.tensor.matmul(
                    ps2[:, 0, :],
                    qk[s][0:DH, 4 + g, jt * P:(jt + 1) * P],
                    qk[s][0:DH, g, :], start=True, stop=True)
                nc.tensor.matmul(
                    ps2[:, 1, :],
                    qk[s][DH:P, 4 + g, jt * P:(jt + 1) * P],
                    qk[s][DH:P, g, :], start=True, stop=True,
                    tile_position=(DH, 0))
                e = espool.tile([P, 2, NSEQ], f32r, tag="es", name="es")
                nc.scalar.activation(
                    e[:].rearrange("p a b -> p (a b)"),
                    ps2[:].rearrange("p a b -> p (a b)"), Exp, scale=SCALE)
                es[(g, jt)] = e

            def av_unit(s, g):
                po0 = psop.tile([DH + 1, NSEQ], f32, tag="pso", name="pso")
                po1 = psop.tile([DH + 1, NSEQ], f32, tag="pso", name="pso")
                for jt in range(4):
                    e = es.pop((g, jt))
                    nc.tensor.matmul(
                        po0[:], va[s][:, jt, 2 * g, :], e[:, 0, :],
                        start=(jt == 0), stop=(jt == 3))
                    nc.tensor.matmul(
                        po1[:], va[s][:, jt, 2 * g + 1, :], e[:, 1, :],
                        start=(jt == 0), stop=(jt == 3))
                return po0, po1

            def evac_unit(s, g, po0, po1):
                """PSUM -> SBUF (rows 0..64 incl. denominator row), then the
                denominator rows -> DRAM scratch for the band recip pass."""
                o = opool.tile([P, 2, NSEQ], f32, tag="oraw", name="oraw")
                nc.scalar.activation(o[0:DH + 1, 0, :], po0[:], Copy)
                nc.vector.tensor_copy(o[0:DH + 1, 1, :], po1[:])
                nc.sync.dma_start(dD[s][2 * g, :], o[DH:DH + 1, 0, :])
                nc.sync.dma_start(dD[s][2 * g + 1, :], o[DH:DH + 1, 1, :])
                oraw[(s, g)] = o

            def recip_unit(s):
                """All 8 heads' 1/denominator, lane-parallel: gather the 8
                [512] rows as [128, 8, 4], 1/d = exp(-ln d), scatter back."""
                dsb = rpool.tile([P, 8, 4], f32, tag="dsb", name="dsb")
                nc.sync.dma_start(
                    dsb[:], dD[s].rearrange("h (c p) -> p h c", p=P))
                lgT = rpool.tile([P, 32], f32, tag="lgT", name="lgT")
                nc.scalar.activation(
                    lgT[:], dsb[:].rearrange("p a b -> p (a b)"), Ln)
                rT = rpool.tile([P, 32], f32, tag="rT", name="rT")
                nc.scalar.activation(rT[:], lgT[:], Exp, scale=-1.0)
                nc.sync.dma_start(
                    rD[s][0].rearrange("h c p -> p (h c)"), rT[:])

            def mult_unit(s, g):
                """ot[:, g, :] = O^T * (1/d) via stride-0 DRAM broadcast."""
                rb = rbpool.tile([DH, 2 * NSEQ], f32, tag="rb", name="rb")
                src = rD[s][0:1, 2 * g:2 * g + 2, :, :].rearrange(
                    "o h c p -> o (h c p)")
                nc.sync.dma_start(rb[:], src.to_broadcast((DH, 2 * NSEQ)))
                o = oraw.pop((s, g))
                nc.vector.tensor_mul(ot[s][0:DH, g, :], o[0:DH, 0, :],
                                     rb[:, 0:NSEQ])
                nc.vector.tensor_mul(ot[s][DH:P, g, :], o[0:DH, 1, :],
                                     rb[:, NSEQ:2 * NSEQ])

            def oproj_unit(s, n):
                ps = psproj.tile([P, NSEQ], f32, tag="psproj", name="psp")
                for kt in range(4):
                    nc.tensor.matmul(
                        ps[:], ot[s][:, kt, n * P:(n + 1) * P], wo_sb[:, kt, :],
                        start=(kt == 0), stop=(kt == 3))
                ob = obpool.tile([P, D], f32, tag="ob", name="ob")
                nc.vector.tensor_add(ob[:], ps[:], bias_sb[:])
                nc.sync.dma_start(
                    out[s].rearrange("(no ni) e -> ni no e", ni=P)[:, n, :],
                    ob[:])

            rep_ctx = (tc.For_i(0, repeat, 1,
                                hint_engines=(mybir.EngineType.PE,
                                              mybir.EngineType.Activation,
                                              mybir.EngineType.DVE))
                       if repeat > 1 else contextlib.nullcontext())
            with rep_ctx:
                load_x(0)
                load_x(1)
                for k in range(12):
                    proj_unit(0, k)

                for s in range(nbands):
                    if s + 2 < nbands:
                        load_x(s + 2)
                    dD[s] = drampool.tile([H, NSEQ], f32, tag="dD", name="dD")
                    rD[s] = drampool.tile([1, H, 4, P], f32, tag="rD",
                                          name="rD")
                    ot[s] = otpool.tile([P, 4, NSEQ], f32r, tag="ot",
                                        name="ot")

                    fillers = []
                    if s + 1 < nbands:
                        fillers += [(proj_unit, (s + 1, k)) for k in range(12)]
                    if s >= 1:
                        fillers += [(oproj_unit, (s - 1, n)) for n in range(4)]
                    fq = iter(fillers)

                    def filler():
                        u = next(fq, None)
                        if u is not None:
                            u[0](*u[1])

                    pending_av = None
                    for g in range(4):
                        for jt in range(4):
                            s_unit(s, g, jt)
                            filler()
                        if g >= 1:
                            po0, po1 = av_unit(s, g - 1)
                            evac_unit(s, g - 1, po0, po1)
                    po0, po1 = av_unit(s, 3)
                    evac_unit(s, 3, po0, po1)
                    # drain unused fillers (bands 0 and 6 have fewer)
                    for u in fq:
                        u[0](*u[1])
                    recip_unit(s)
                    for g in range(4):
                        mult_unit(s, g)
                    xt.pop(s, None)

                for n in range(4):
                    oproj_unit(nbands - 1, n)
                qk.clear(); va.clear(); ot.clear()
                dD.clear(); rD.clear(); xt.clear()

    nc.compile()
    return nc


def _get_nc():
    global _cached
    if _cached is None:
        _cached = build_kernel()
    return _cached


def make_in_maps(x, x_delta, x_theta, x_alpha, x_beta, x_gamma, x_upper,
                 Wqkv, Wout, bout, mm_dtype=MM_DTYPE):
    if mm_dtype == "f32r":
        cast_dt = np.float32
    else:
        import ml_dtypes
        cast_dt = ml_dtypes.bfloat16
    xs = np.stack([np.asarray(a, dtype=np.float32) for a in
                   (x, x_delta, x_theta, x_alpha, x_beta, x_gamma, x_upper)],
                  axis=0)  # [7, b, n, d]
    xsT = np.ascontiguousarray(xs.transpose(1, 0, 3, 2).astype(cast_dt))
    wqkvT = np.ascontiguousarray(np.asarray(Wqkv, np.float32).T.astype(cast_dt))
    woutT = np.ascontiguousarray(np.asarray(Wout, np.float32).T.astype(cast_dt))
    biasb = np.ascontiguousarray(
        np.broadcast_to(np.asarray(bout, np.float32)[None, :], (P, D)))
    return [
        {"xT": xsT[c], "wqkvT": wqkvT, "woutT": woutT, "biasb": biasb}
        for c in range(NCORES)
    ]


def kernel(x, x_delta, x_theta, x_alpha, x_beta, x_gamma, x_upper,
           Wqkv, Wout, bout):
    from concourse.bass_utils import run_bass_kernel_spmd

    nc = _get_nc()
    in_maps = make_in_maps(x, x_delta, x_theta, x_alpha, x_beta, x_gamma,
                           x_upper, Wqkv, Wout, bout)
    res = run_bass_kernel_spmd(nc, in_maps, core_ids=list(range(NCORES)))
    full = np.empty((NBANDS, NCORES, NSEQ, D), dtype=np.float32)
    for c in range(NCORES):
        full[:, c] = res.results[c]["out"]
    return tuple(full[i] for i in range(NBANDS))


# revision 22
# speedup vs baseline: 5.8618x; 1.5892x over previous
"""Trainium2 Bass kernel for nn_Attention_86268713108190.

7 independent attention "bands" over batch 8, n=512, d=512, 8 heads,
shared Wqkv/Wout. Sharding: data-parallel over batch — core c handles
batch index c (7 band-samples of [512, 512] each).

v2: software-pipelined band schedule built around keeping the PE
(tensor engine) continuously fed so the HAM clock gate stays at
2.4 GHz (it throttles to 1.2 GHz after ~3.4 us of idle):

  - Per band: QKV/V projections (12 groups x 4 matmuls, f32r), S^T
    matmuls per head pair as row-tiled concurrent pairs
    (tile_position (0,0)/(64,0), K=64), batched exp on ACT over a
    2-bank PSUM tile [128, 2x512], AV with a ones column appended to V
    so the softmax denominator falls out of the AV matmul (row 64).
  - The attention phase is ACT-paced (exp), so the NEXT band's
    projection matmuls and the PREVIOUS band's out-projection are
    interleaved as fillers between S units to keep PE busy.
  - Softmax normalize: AV PSUM is evacuated to SBUF ([65, 512] copies,
    split ACT/DVE), denominator rows DMA to a DRAM scratch, ONE
    [128, 8x4] gather + Ln + Exp(scale=-1) computes all 8 heads'
    reciprocals per band in lane-parallel row-major form (instead of
    lane-starved [1, N] ops), then a stride-0 DRAM broadcast DMA and
    2 DVE multiplies per pair normalize O^T. Out-projection of band s
    runs as filler inside band s+1, so nothing on the PE ever waits
    for the normalize chain.

Whole-output HW accuracy vs fp32 reference: rel err ~3e-4 (f32r
matmuls everywhere, same numerics as v1).
"""

import contextlib
import sys

if '/opt/trn_rl_repo' not in sys.path:
    sys.path.insert(0, '/opt/trn_rl_repo')

import numpy as np

P = 128
MM_DTYPE = "f32r"
NSEQ = 512
D = 512
H = 8
DH = 64
NBANDS = 7
NCORES = 8
SCALE = D ** -0.5

_cached = None


def build_kernel(nbands=NBANDS, repeat=1, mm_dtype=MM_DTYPE, tail="free",
                 expmode="split"):
    import concourse.mybir as mybir
    import concourse.tile as tile
    from concourse import bacc
    from concourse import library_config

    f32 = mybir.dt.float32
    f32r = (mybir.dt.float32r if mm_dtype == "f32r" else mybir.dt.bfloat16)
    Exp = mybir.ActivationFunctionType.Exp
    Ln = mybir.ActivationFunctionType.Ln
    Copy = mybir.ActivationFunctionType.Copy

    nc = bacc.Bacc("TRN2", target_bir_lowering=False, debug=False,
                   num_devices=NCORES)

    xT = nc.dram_tensor("xT", [nbands, D, NSEQ], f32r, kind="ExternalInput").ap()
    wqkvT = nc.dram_tensor("wqkvT", [D, 3 * D], f32r, kind="ExternalInput").ap()
    woutT = nc.dram_tensor("woutT", [D, D], f32r, kind="ExternalInput").ap()
    biasb = nc.dram_tensor("biasb", [P, D], f32, kind="ExternalInput").ap()
    out = nc.dram_tensor("out", [nbands, NSEQ, D], f32, kind="ExternalOutput").ap()

    nc.gpsimd.load_library(library_config.attn)

    with tile.TileContext(nc) as tc:
        with (
            tc.tile_pool(name="weights", bufs=1) as wpool,
            tc.tile_pool(name="x", bufs=3) as xpool,
            tc.tile_pool(name="qk", bufs=2) as qkpool,
            tc.tile_pool(name="v", bufs=2) as vpool,
            tc.tile_pool(name="ot", bufs=2) as otpool,
            tc.tile_pool(name="es", bufs=8) as espool,
            tc.tile_pool(name="oraw", bufs=5) as opool,
            tc.tile_pool(name="r", bufs=2) as rpool,
            tc.tile_pool(name="rb", bufs=3) as rbpool,
            tc.tile_pool(name="ob", bufs=3) as obpool,
            tc.tile_pool(name="dram", bufs=2, space="DRAM") as drampool,
            tc.tile_pool(name="psproj", bufs=2, space="PSUM") as psproj,
            tc.tile_pool(name="pss", bufs=2, space="PSUM") as pssp,
            tc.tile_pool(name="pso", bufs=2, space="PSUM") as psop,
        ):
            wq_sb = wpool.tile([P, 4, 3 * D], f32r)
            wo_sb = wpool.tile([P, 4, D], f32r)
            bias_sb = wpool.tile([P, D], f32)
            # band-end recip scratch: pair g's [1, 1024] d-row pair lives in
            # partition-row 32*g (GPSIMD cores own 16-partition slices, so
            # its writes must start on those boundaries), making the whole
            # band's reciprocal ONE lane-parallel Ln + ONE Exp (2 ACT table
            # loads per band instead of 2 per pair; a load is 1283 ns)
            dball = wpool.tile([3 * 32 + 1, 2 * NSEQ], f32)
            lgS = wpool.tile([3 * 32 + 1, 2 * NSEQ], f32)
            rccS = wpool.tile([3 * 32 + 1, 2 * NSEQ], f32)
            nc.vector.memset(dball[:], 1.0)
            wq_r = wqkvT.rearrange("(ko ki) e -> ki ko e", ki=P)
            for kt in range(4):
                nc.sync.dma_start(wq_sb[:, kt, :], wq_r[:, kt, :])
            nc.sync.dma_start(wo_sb[:], woutT.rearrange("(ko ki) e -> ki ko e", ki=P))
            nc.sync.dma_start(bias_sb[:], biasb[:])

            # per-band live tiles (keyed by band index)
            xt = {}
            qk = {}
            va = {}
            ot = {}
            oraw = {}
            es = {}
            dD = {}
            rD = {}
            drs = {}

            ET_ORDER = (0, 4, 1, 5, 2, 6, 3, 7)

            def load_x(s):
                t = xpool.tile([P, 4, NSEQ], f32r, tag="xt", name="xt")
                nc.sync.dma_start(
                    t[:], xT[s].rearrange("(ko ki) n -> ki ko n", ki=P))
                xt[s] = t

            def proj_unit(s, k):
                """k in 0..7: q,k column groups; k in 8..11: v row groups."""
                if k == 0:
                    qk[s] = qkpool.tile([P, 8, NSEQ], f32r, tag="qk", name="qk")
                if k == 8:
                    va[s] = vpool.tile([P, 4, H, DH + 1], f32r, tag="va",
                                       name="va")
                if k < 8:
                    et = ET_ORDER[k]
                    ps = psproj.tile([P, NSEQ], f32, tag="psproj", name="psp")
                    for kt in range(4):
                        nc.tensor.matmul(
                            ps[:], wq_sb[:, kt, et * P:(et + 1) * P],
                            xt[s][:, kt, :], start=(kt == 0), stop=(kt == 3))
                    nc.vector.tensor_copy(qk[s][:, et, :], ps[:])
                else:
                    nt = k - 8
                    ps = psproj.tile([P, NSEQ], f32, tag="psproj", name="psp")
                    for kt in range(4):
                        nc.tensor.matmul(
                            ps[:], xt[s][:, kt, nt * P:(nt + 1) * P],
                            wq_sb[:, kt, 2 * D:3 * D],
                            start=(kt == 0), stop=(kt == 3))
                    nc.vector.tensor_copy(
                        va[s][:, nt, :, 0:DH],
                        ps[:].rearrange("p (h dh) -> p h dh", h=H))
                    ones_slice = va[s][:, nt, :, DH:DH + 1]
                    if mm_dtype == "f32r":
                        ones_slice = ones_slice.bitcast(f32)
                    nc.vector.memset(ones_slice, 1.0)

            def s_unit(s, g, jt):
                """S^T for head pair (2g, 2g+1), j-tile jt, + batched exp."""
                ps2 = pssp.tile([P, 2, NSEQ], f32, tag="pss", name="pss")
                nc.tensor.matmul(
                    ps2[:, 0, :],
                    qk[s][0:DH, 4 + g, jt * P:(jt + 1) * P],
                    qk[s][0:DH, g, :], start=True, stop=True)
                nc.tensor.matmul(
                    ps2[:, 1, :],
                    qk[s][DH:P, 4 + g, jt * P:(jt + 1) * P],
                    qk[s][DH:P, g, :], start=True, stop=True,
                    tile_position=(DH, 0))
                e = espool.tile([P, 2, NSEQ], f32r, tag="es", name="es")
                if expmode == "batched":
                    nc.scalar.activation(
                        e[:].rearrange("p a b -> p (a b)"),
                        ps2[:].rearrange("p a b -> p (a b)"), Exp, scale=SCALE)
                else:
                    nc.scalar.activation(e[:, 0, :], ps2[:, 0, :], Exp,
                                         scale=SCALE)
                    nc.scalar.activation(e[:, 1, :], ps2[:, 1, :], Exp,
                                         scale=SCALE)
                es[(g, jt)] = e

            def av_unit(s, g):
                po0 = psop.tile([DH + 1, NSEQ], f32, tag="pso", name="pso")
                po1 = psop.tile([DH + 1, NSEQ], f32, tag="pso", name="pso")
                for jt in range(4):
                    e = es.pop((g, jt))
                    nc.tensor.matmul(
                        po0[:], va[s][:, jt, 2 * g, :], e[:, 0, :],
                        start=(jt == 0), stop=(jt == 3))
                    nc.tensor.matmul(
                        po1[:], va[s][:, jt, 2 * g + 1, :], e[:, 1, :],
                        start=(jt == 0), stop=(jt == 3))
                return po0, po1

            def evac_unit(s, g, po0, po1):
                """PSUM -> SBUF (rows 0..64 incl. denominator row), then kick
                off this pair's reciprocal chain (tail-variant dependent)."""
                if tail == "none":
                    nc.scalar.activation(ot[s][0:DH, g, :], po0[0:DH, :], Copy)
                    nc.vector.tensor_copy(ot[s][DH:P, g, :], po1[0:DH, :])
                    return
                o = opool.tile([P, 2, NSEQ], f32, tag="oraw", name="oraw")
                nc.vector.tensor_copy(o[0:DH + 1, 0, :], po0[:])
                nc.vector.tensor_copy(o[0:DH + 1, 1, :], po1[:])
                oraw[(s, g)] = o
                if tail == "free":
                    collect_d(s, g)
                if tail == "row":
                    nc.sync.dma_start(dD[s][2 * g, :], o[DH:DH + 1, 0, :])
                    nc.sync.dma_start(dD[s][2 * g + 1, :], o[DH:DH + 1, 1, :])

            def recip_unit(s):
                """tail == "row" only: all 8 heads' 1/denominator,
                lane-parallel: gather the 8 [512] rows as [128, 8, 4],
                1/d = exp(-ln d), scatter back."""
                if tail != "row":
                    return
                dsb = rpool.tile([P, 8, 4], f32, tag="dsb", name="dsb")
                nc.sync.dma_start(
                    dsb[:], dD[s].rearrange("h (c p) -> p h c", p=P))
                lgT = rpool.tile([P, 32], f32, tag="lgT", name="lgT")
                nc.scalar.activation(
                    lgT[:], dsb[:].rearrange("p a b -> p (a b)"), Ln)
                rT = rpool.tile([P, 32], f32, tag="rT", name="rT")
                nc.scalar.activation(rT[:], lgT[:], Exp, scale=-1.0)
                nc.sync.dma_start(
                    rD[s][0].rearrange("h c p -> p (h c)"), rT[:])

            def collect_d(s, g):
                """Move pair g's [1, 1024] denominator row-pair into
                partition-row g of dball, on the otherwise-idle GPSIMD."""
                o = oraw[(s, g)]
                nc.gpsimd.tensor_copy(
                    dball[32 * g:32 * g + 1, :],
                    o[DH:DH + 1, :, :].rearrange("p a b -> p (a b)"))

            def tail_free_units(s):
                """Band-end reciprocal: one [4, 1024] Ln + one Exp(-x)."""
                nc.scalar.activation(lgS[:], dball[:], Ln)
                nc.scalar.activation(rccS[:], lgS[:], Exp, scale=-1.0)
                for g in range(4):
                    dr = drampool.tile([1, 2 * NSEQ], f32, tag="dr", name="dr",
                                       bufs=6)
                    nc.sync.dma_start(dr[:], rccS[32 * g:32 * g + 1, :])
                    drs[(s, g)] = dr

            def mult_unit(s, g):
                """ot[:, g, :] = O^T * (1/d) via stride-0 DRAM broadcast."""
                if tail == "none":
                    return
                rb = rbpool.tile([DH, 2 * NSEQ], f32, tag="rb", name="rb")
                if tail == "row":
                    src = rD[s][0:1, 2 * g:2 * g + 2, :, :].rearrange(
                        "o h c p -> o (h c p)")
                    nc.sync.dma_start(rb[:], src.to_broadcast((DH, 2 * NSEQ)))
                else:
                    dr = drs.pop((s, g))
                    nc.sync.dma_start(rb[:],
                                      dr[:].to_broadcast((DH, 2 * NSEQ)))
                o = oraw.pop((s, g))
                nc.vector.tensor_mul(ot[s][0:DH, g, :], o[0:DH, 0, :],
                                     rb[:, 0:NSEQ])
                nc.vector.tensor_mul(ot[s][DH:P, g, :], o[0:DH, 1, :],
                                     rb[:, NSEQ:2 * NSEQ])

            def oproj_unit(s, n):
                ps = psproj.tile([P, NSEQ], f32, tag="psproj", name="psp")
                for kt in range(4):
                    nc.tensor.matmul(
                        ps[:], ot[s][:, kt, n * P:(n + 1) * P], wo_sb[:, kt, :],
                        start=(kt == 0), stop=(kt == 3))
                ob = obpool.tile([P, D], f32, tag="ob", name="ob")
                nc.vector.tensor_add(ob[:], ps[:], bias_sb[:])
                nc.sync.dma_start(
                    out[s].rearrange("(no ni) e -> ni no e", ni=P)[:, n, :],
                    ob[:])

            rep_ctx = (tc.For_i(0, repeat, 1,
                                hint_engines=(mybir.EngineType.PE,
                                              mybir.EngineType.Activation,
                                              mybir.EngineType.DVE))
                       if repeat > 1 else contextlib.nullcontext())
            with rep_ctx:
                load_x(0)
                load_x(1)
                for k in range(12):
                    proj_unit(0, k)

                for s in range(nbands):
                    if s + 2 < nbands:
                        load_x(s + 2)
                    if tail == "row":
                        dD[s] = drampool.tile([H, NSEQ], f32, tag="dD",
                                              name="dD")
                        rD[s] = drampool.tile([1, H, 4, P], f32, tag="rD",
                                              name="rD")
                    ot[s] = otpool.tile([P, 4, NSEQ], f32r, tag="ot",
                                        name="ot")

                    fillers = []
                    if s + 1 < nbands:
                        fillers += [(proj_unit, (s + 1, k)) for k in range(12)]
                    if s >= 1:
                        fillers += [(oproj_unit, (s - 1, n)) for n in range(4)]
                    fq = iter(fillers)

                    def filler():
                        u = next(fq, None)
                        if u is not None:
                            u[0](*u[1])

                    pending_av = None
                    for g in range(4):
                        for jt in range(4):
                            s_unit(s, g, jt)
                            filler()
                        if g >= 1:
                            po0, po1 = av_unit(s, g - 1)
                            evac_unit(s, g - 1, po0, po1)
                    po0, po1 = av_unit(s, 3)
                    evac_unit(s, 3, po0, po1)
                    # drain unused fillers (bands 0 and 6 have fewer)
                    for u in fq:
                        u[0](*u[1])
                    if tail == "free":
                        tail_free_units(s)
                    recip_unit(s)
                    for g in range(4):
                        mult_unit(s, g)
                    xt.pop(s, None)

                for n in range(4):
                    oproj_unit(nbands - 1, n)
                qk.clear(); va.clear(); ot.clear()
                dD.clear(); rD.clear(); xt.clear()

    nc.compile()
    return nc


def _get_nc():
    global _cached
    if _cached is None:
        _cached = build_kernel()
    return _cached


def make_in_maps(x, x_delta, x_theta, x_alpha, x_beta, x_gamma, x_upper,
                 Wqkv, Wout, bout, mm_dtype=MM_DTYPE):
    if mm_dtype == "f32r":
        cast_dt = np.float32
    else:
        import ml_dtypes
        cast_dt = ml_dtypes.bfloat16
    xs = np.stack([np.asarray(a, dtype=np.float32) for a in
                   (x, x_delta, x_theta, x_alpha, x_beta, x_gamma, x_upper)],
                  axis=0)  # [7, b, n, d]
    xsT = np.ascontiguousarray(xs.transpose(1, 0, 3, 2).astype(cast_dt))
    wqkvT = np.ascontiguousarray(np.asarray(Wqkv, np.float32).T.astype(cast_dt))
    woutT = np.ascontiguousarray(np.asarray(Wout, np.float32).T.astype(cast_dt))
    biasb = np.ascontiguousarray(
        np.broadcast_to(np.asarray(bout, np.float32)[None, :], (P, D)))
    return [
        {"xT": xsT[c], "wqkvT": wqkvT, "woutT": woutT, "biasb": biasb}
        for c in range(NCORES)
    ]


def kernel(x, x_delta, x_theta, x_alpha, x_beta, x_gamma, x_upper,
           Wqkv, Wout, bout):
    from concourse.bass_utils import run_bass_kernel_spmd

    nc = _get_nc()
    in_maps = make_in_maps(x, x_delta, x_theta, x_alpha, x_beta, x_gamma,
                           x_upper, Wqkv, Wout, bout)
    res = run_bass_kernel_spmd(nc, in_maps, core_ids=list(range(NCORES)))
    full = np.empty((NBANDS, NCORES, NSEQ, D), dtype=np.float32)
    for c in range(NCORES):
        full[:, c] = res.results[c]["out"]
    return tuple(full[i] for i in range(NBANDS))
